# revision 14
# baseline (speedup 1.0000x reference)
"""E3Hamiltonian spin projection kernel for Trainium2 (Bass/Tile).

The reference op packs 8 real channels into 4 complex (0,y,z,x) channels,
applies a fixed 4x4 complex spin-projection matrix M/sqrt(2), and unpacks
back to real storage.  Expanded to real arithmetic it is 4 butterflies per
spatial position:

    OUT[0] = k*(IN0 + IN2)   OUT[3] = k*(IN0 - IN2)
    OUT[4] = k*(IN4 + IN6)   OUT[7] = k*(IN4 - IN6)
    OUT[1] = k*(IN3 + IN5)   OUT[2] = k*(IN3 - IN5)
    OUT[6] = k*(IN1 + IN7)   OUT[5] = k*(IN7 - IN1)

with k = 1/sqrt(2), applied over every (batch, l, r) position.  Pure
memory-bound streaming: shard batch across 8 cores, stream [128, 4*1352]
tiles, prescale by k on ScalarE, 8 VectorE add/sub ops per tile.
"""

import math

import numpy as np

import concourse.bacc as bacc
import concourse.mybir as mybir
import concourse.tile as tile
from concourse.bass_utils import run_bass_kernel_spmd

B, C, NL, NR = 65536, 8, 13, 13
M = NL * NR            # 169 spatial positions per channel
ROW = C * M            # 1352 floats per batch row
N_CORES = 8
B_LOC = B // N_CORES   # 8192 batch rows per core
P = 128                # SBUF partitions
G = 4                  # 128-batch groups per tile
N_TILES = B_LOC // (P * G)
K = 1.0 / math.sqrt(2.0)

# (a, b, sum_out, diff_out): OUT[sum_out] = k*(IN[a]+IN[b]), OUT[diff_out] = k*(IN[a]-IN[b])
BUTTERFLIES = [
    (0, 2, 0, 3),
    (4, 6, 4, 7),
    (3, 5, 1, 2),
    (7, 1, 6, 5),
]

_cache = {}


def build_bass(b_loc=B_LOC, loop_repeats=1, split_rings=False, bufs=2, g=8,
               body_mult=1, swdge_out=False, pg_order=True):
    n_tiles = b_loc // (P * g) * body_mult
    nc = bacc.Bacc("TRN2", target_bir_lowering=False, debug=False)
    f32 = mybir.dt.float32
    x = nc.dram_tensor("x", [b_loc, ROW], f32, kind="ExternalInput")
    y = nc.dram_tensor("y", [b_loc, ROW], f32, kind="ExternalOutput")
    if pg_order:
        # partition p holds g consecutive batch rows -> one contiguous
        # g*ROW*4-byte run per partition per DMA
        xv = x[:].rearrange("(n p g) m -> n p g m", g=g, p=P)
        yv = y[:].rearrange("(n p g) m -> n p g m", g=g, p=P)
    else:
        xv = x[:].rearrange("(n g p) m -> n p g m", g=g, p=P)
        yv = y[:].rearrange("(n g p) m -> n p g m", g=g, p=P)
    n_slices = b_loc // (P * g)

    with tile.TileContext(nc) as tc:
        store_eng = nc.gpsimd if swdge_out else (nc.scalar if split_rings else nc.sync)
        with (
            tc.tile_pool(name="tin", bufs=bufs) as in_pool,
            tc.tile_pool(name="tout", bufs=bufs) as out_pool,
        ):
            def body():
                for t in range(n_tiles):
                    t = t % n_slices
                    tin = in_pool.tile([P, g * ROW], f32)
                    tin3 = tin[:].rearrange("p (g m) -> p g m", g=g)
                    nc.sync.dma_start(tin3, xv[t])
                    nc.scalar.mul(tin[:], tin[:], K)
                    tout = out_pool.tile([P, g * ROW], f32)
                    tout3 = tout[:].rearrange("p (g m) -> p g m", g=g)
                    for a, b, so, do in BUTTERFLIES:
                        ina = tin3[:, :, a * M:(a + 1) * M]
                        inb = tin3[:, :, b * M:(b + 1) * M]
                        nc.vector.tensor_add(tout3[:, :, so * M:(so + 1) * M], ina, inb)
                        nc.vector.tensor_sub(tout3[:, :, do * M:(do + 1) * M], ina, inb)
                    store_eng.dma_start(yv[t], tout3)

            if loop_repeats == 1:
                body()
            else:
                with tc.For_i(0, loop_repeats, 1):
                    body()
    nc.compile()
    return nc


def kernel(HR_in: np.ndarray) -> np.ndarray:
    flat = np.ascontiguousarray(HR_in, dtype=np.float32).reshape(B, ROW)
    in_maps = [{"x": flat[i * B_LOC:(i + 1) * B_LOC]} for i in range(N_CORES)]
    nc = _cache.get("nc")
    if nc is None:
        nc = _cache["nc"] = build_bass()
    res = run_bass_kernel_spmd(nc, in_maps, core_ids=list(range(N_CORES)))
    out = np.concatenate([r["y"] for r in res.results], axis=0)
    return out.reshape(B, C, NL, NR)
